# revision 18
# baseline (speedup 1.0000x reference)
"""Bahdanau-attention kernel for one TRN2 chip (8 NeuronCores, SPMD).

Math (per batch row b, sequence position s):
    att[b, s] = v . tanh(h_part[b] + enc[s, b, :] @ W_e)
    out[b, :] = softmax(att[b, :])        with h_part = hidden @ W_h + b_attn

Sharding: pure data-parallel over batch (B=32 -> 4 per core), no collectives.

v2 design notes (from the v1 trace: 79.6us, ACT busy 63us, first ACT op at
t=24.5us, PE at half clock for the last 16us):
- Prologue cut from 24.5us to ~5us: weights arrive on parallel DMA queues
  (sync/vector/gpsimd), h_part runs as fp8 DoubleRow (8 matmuls) right after
  a short PE clock warm-up, and tanh of block 0 is gated only on its own
  e-matmul + the h_part transpose chain.
- s-major block order (all 4 batch rows at seq-block 0, then seq-block 1).
- The v-dot for (row b, 512-chunk c) uses a zero-padded stationary [128, 32]
  with v in column c, so its logit lands on psum partition 32b+c.  All 16
  (b, c) logit vectors accumulate into ONE [128, 512] psum bank; strips are
  per-b accumulation groups spanning all 4 chunks (zeros elsewhere add 0).
- Softmax then collapses to: ONE [128, 512] exp (+accum row sums), a [128,4]
  selection matmul for per-row sums, reciprocal, a broadcast-back matmul to
  per-partition scalars, one DVE tensor_scalar multiply, and ONE [128, 512]
  output DMA (2KB/partition instead of 8KB on a single partition).
- The ACT queue carries only 32 tanh + 1 exp; all DMAs are issued from
  sync/vector/gpsimd queues (v1 burned 6.5us of ACT time on DMA_DIRECT2D),
  and softmax reductions/normalization run on DVE/PE, not ACT.
Measured v1: 79.6us.  This file: see test output.
"""

import sys

sys.path.insert(0, "/opt/trn_rl_repo")

import numpy as np

from concourse import bacc, bass, mybir, tile
from concourse.bass_utils import run_bass_kernel_spmd

H = 512
DH = 4 * H            # 2048 (hidden feature dim)
B, S = 32, 2048
NCORES = 8
BC = B // NCORES      # 4 batch rows per core
KH = H // 128         # 4 contraction tiles over H
KD = DH // 128        # 16 contraction tiles over DH
NQ = H // 128         # 4 output quadrants of H
SBLK = 1024           # sequence positions per block
NBLK = S // SBLK      # 2 seq blocks per batch row
HB = 512              # half-block: psum-bank / matmul-N granularity
NCH = S // HB         # 4 512-chunks per row (global chunk index c)
F32 = mybir.dt.float32
BF16 = mybir.dt.bfloat16
F8 = mybir.dt.float8e4
WE_SCALE = 64.0
WH_SCALE = 512.0

_NC_CACHE = None


def _build():
    nc = bacc.Bacc(
        "TRN2", target_bir_lowering=False, debug=False, num_devices=NCORES
    )
    enc_d = nc.dram_tensor(
        "enc_t", [BC, NBLK, 128, KH, SBLK], F8, kind="ExternalInput"
    )
    hid_d = nc.dram_tensor("hid_t", [128, KD, 16], F8, kind="ExternalInput")
    wh_d = nc.dram_tensor("w_h", [128, KD, H], F8, kind="ExternalInput")
    we_d = nc.dram_tensor("w_e", [128, KH, H], F8, kind="ExternalInput")
    ba_d = nc.dram_tensor("ba4", [128, NQ, BC], F32, kind="ExternalInput")
    vs_d = nc.dram_tensor("vs32", [128, NQ, NCH, 32], BF16, kind="ExternalInput")
    m4_d = nc.dram_tensor("m4", [128, BC], F32, kind="ExternalInput")
    m4t_d = nc.dram_tensor("m4t", [BC, 128], F32, kind="ExternalInput")
    id_d = nc.dram_tensor("ident", [BC, BC], F32, kind="ExternalInput")
    out_d = nc.dram_tensor("out", [128, HB], BF16, kind="ExternalOutput")

    TANH = mybir.ActivationFunctionType.Tanh
    EXP = mybir.ActivationFunctionType.Exp

    with tile.TileContext(nc) as tc:
        with (
            tc.tile_pool(name="const", bufs=1) as constp,
            tc.tile_pool(name="enc", bufs=4) as encp,
            tc.tile_pool(name="energy", bufs=3) as enp,
            tc.tile_pool(name="small", bufs=1) as smallp,
            tc.tile_pool(name="psum_e", bufs=3, space=bass.MemorySpace.PSUM) as pse,
            tc.tile_pool(name="psum_l", bufs=1, space=bass.MemorySpace.PSUM) as psl,
            tc.tile_pool(name="psum_s", bufs=1, space=bass.MemorySpace.PSUM) as pss,
        ):
            # ---- DMAs: measured per-queue rates sync ~100GB/s, scalar ~60,
            # gpsimd ~40 (one ~350GB/s/core HBM slice shared by 8 cores).
            # enc block 0 (sync) and we (scalar) go first -- they gate the
            # first e-matmul; wh (1MB, gating the tanh bias) is spread over
            # all three queues to land ~simultaneously; later enc blocks on
            # sync only, so no DMA issue slot interrupts the ACT stream. ----
            we_sb = constp.tile([128, KH, H], F8)
            nc.scalar.dma_start(we_sb[:], we_d[:])
            hid_sb = constp.tile([128, KD, 16], F8)
            nc.gpsimd.dma_start(hid_sb[:], hid_d[:])
            ba_sb = constp.tile([128, NQ, BC], F32)
            nc.gpsimd.dma_start(ba_sb[:], ba_d[:])
            id_sb = constp.tile([BC, BC], F32)
            nc.gpsimd.dma_start(id_sb[:], id_d[:])
            m4_sb = constp.tile([128, BC], F32)
            nc.gpsimd.dma_start(m4_sb[:], m4_d[:])
            m4t_sb = constp.tile([BC, 128], F32)
            nc.gpsimd.dma_start(m4t_sb[:], m4t_d[:])
            et0 = encp.tile([128, KH, SBLK], F8)
            nc.sync.dma_start(et0[:], enc_d[0, 0])
            wh_sb = constp.tile([128, KD, H], F8)
            nc.gpsimd.dma_start(wh_sb[:, 12:16, :], wh_d[:, 12:16, :])
            nc.scalar.dma_start(wh_sb[:, 6:12, :], wh_d[:, 6:12, :])
            nc.sync.dma_start(wh_sb[:, 0:6, :], wh_d[:, 0:6, :])
            vs_sb = constp.tile([128, NQ, NCH, 32], BF16)
            nc.scalar.dma_start(vs_sb[:], vs_d[:])

            hptb = constp.tile([128, NQ, BC], F32)
            ex = smallp.tile([128, HB], F32)
            out_sb = smallp.tile([128, HB], BF16)
            esum = smallp.tile([128, 1], F32)
            rsb = smallp.tile([BC, 1], F32)
            hp_sb = smallp.tile([BC, H], F32)

            logit_ps = psl.tile([128, HB], F32)
            ps_t = pss.tile([128, HB], F32)

            # ---- PE clock warm-up on dummy data while DMAs fly ----
            warm = constp.tile([128, 512], BF16)
            nc.vector.memset(warm[:], 0.0)
            for _ in range(10):
                nc.tensor.matmul(
                    ps_t[:, :], warm[:, 0:128], warm[:], start=True, stop=True
                )

            # ---- h_part = hidden @ W_h (fp8 DoubleRow, 8 matmuls).
            # DR ldweights needs plane stride %16==0, so hid is padded to 16
            # columns and hp lands on psum partitions 0:16 (4 real).  Pairs
            # are emitted in wh-chunk arrival order (gpsimd chunk lands
            # first) so partial hp overlaps the trailing wh DMA; emitted
            # after e-matmul block 0, which is ready ~5us earlier. ----
            def emit_hp():
                hp_ps = ps_t[0:16, 0:H]
                hp_order = [3, 4, 0, 5, 1, 6, 2, 7]
                for n, j in enumerate(hp_order):
                    nc.tensor.matmul(
                        hp_ps,
                        hid_sb[:, 2 * j : 2 * j + 2, :],
                        wh_sb[:, 2 * j : 2 * j + 2, :],
                        start=(n == 0),
                        stop=(n == len(hp_order) - 1),
                        perf_mode=mybir.MatmulPerfMode.DoubleRow,
                    )
                nc.vector.tensor_copy(hp_sb[:], ps_t[0:BC, 0:H])
                # transpose to [128, q, b]; fold in 1/WH_SCALE and b_attn
                for q in range(NQ):
                    hpt_ps = ps_t[:, (q + 4) * BC : (q + 5) * BC]
                    nc.tensor.transpose(
                        hpt_ps, hp_sb[:, q * 128 : (q + 1) * 128], id_sb[:]
                    )
                    nc.vector.scalar_tensor_tensor(
                        hptb[:, q, :],
                        hpt_ps,
                        1.0 / WH_SCALE,
                        ba_sb[:, q, :],
                        op0=mybir.AluOpType.mult,
                        op1=mybir.AluOpType.add,
                    )

            # ---- pipeline over blocks, s-major ----
            blocks = [(b, s) for s in range(NBLK) for b in range(BC)]
            NBLOCKS = len(blocks)
            ets = {}
            epss = {}
            ens = {}

            def load_block(i):
                if i == 0:
                    ets[0] = et0
                    return
                b, sblk = blocks[i]
                et = encp.tile([128, KH, SBLK], F8)
                nc.sync.dma_start(et[:], enc_d[b, sblk])
                ets[i] = et

            def emit_emm(i):
                # q-SEQUENTIAL emission: q0's psum completes after 4 matmuls
                # (not 16), so the next tanh starts ~2.6us earlier, and the
                # q3 tile's buffer WAR (it reuses q0's buffer -- 4 tiles per
                # block rotate through 3 psum buffers) resolves while the PE
                # is still busy with q1/q2.  With pair-interleaved emission
                # the serial loop tanh(q0) -> e-mm q2q3 -> v-dots -> e-mm
                # q0q1 -> tanh was 6.3us/block; this cuts it to PE-dense.
                et = ets[i]
                tiles = []
                for q in range(NQ):
                    tiles.append(pse.tile([128, SBLK], F32, name="eps", tag="eps"))
                for q in range(NQ):
                    for half in range(SBLK // HB):
                        hsl = slice(half * HB, (half + 1) * HB)
                        for j in range(KH // 2):
                            nc.tensor.matmul(
                                tiles[q][:, hsl],
                                we_sb[
                                    :, 2 * j : 2 * j + 2, q * 128 : (q + 1) * 128
                                ],
                                et[:, 2 * j : 2 * j + 2, hsl],
                                start=(j == 0),
                                stop=(j == KH // 2 - 1),
                                perf_mode=mybir.MatmulPerfMode.DoubleRow,
                            )
                epss[i] = tiles
                ets.pop(i)

            def emit_tanh(i):
                b, sblk = blocks[i]
                en = enp.tile([128, NQ, SBLK], BF16)
                for q in range(NQ):
                    nc.scalar.activation(
                        en[:, q, :],
                        epss[i][q][:],
                        TANH,
                        bias=hptb[:, q, b : b + 1],
                        scale=1.0 / WE_SCALE,
                    )
                ens[i] = en
                del epss[i]

            def emit_v(i):
                # logits for (row b, global chunk c) -> psum partition 32b+c.
                # Aligned 32-wide zero-padded stationary (v in column c) so
                # LDWEIGHTS stays on an aligned address (an unaligned sliding
                # window slowed every PE issue slot by ~50ns); the strip base
                # partition 32b needs an explicit tile_position since the AP
                # helper rejects base 96.  One accumulation group per strip,
                # spanning all 4 chunks: padding columns only ever add zeros.
                b, sblk = blocks[i]
                en = ens[i]
                strip = logit_ps[32 * b : 32 * b + 32, :]
                for half in range(SBLK // HB):
                    c = sblk * (SBLK // HB) + half
                    hsl = slice(half * HB, (half + 1) * HB)
                    for q in range(NQ):
                        nc.tensor.matmul(
                            strip,
                            vs_sb[:, q, c, :],
                            en[:, q, hsl],
                            start=(sblk == 0 and half == 0 and q == 0),
                            stop=(sblk == NBLK - 1
                                  and half == SBLK // HB - 1 and q == NQ - 1),
                            tile_position=(0, 32 * b),
                        )
                del ens[i]

            load_block(0)
            load_block(1)
            emit_emm(0)
            emit_hp()
            emit_tanh(0)
            for i in range(1, NBLOCKS):
                if i + 1 < NBLOCKS:
                    load_block(i + 1)
                emit_emm(i)
                emit_tanh(i)
                emit_v(i - 1)
            # dummy matmuls run while ACT finishes the last tanh batch, so
            # HAM doesn't halve the PE clock before the final v-dots
            for _ in range(9):
                nc.tensor.matmul(
                    ps_t[:, :], warm[:, 0:128], warm[:], start=True, stop=True
                )
            emit_v(NBLOCKS - 1)

            # ---- softmax tail ----
            nc.scalar.activation(ex[:], logit_ps[:], EXP, accum_out=esum[:])
            rs4_ps = ps_t[0:BC, 0:1]
            nc.tensor.matmul(rs4_ps, m4_sb[:], esum[:], start=True, stop=True)
            nc.vector.reciprocal(rsb[:], rs4_ps)
            rsB_ps = ps_t[:, 4:5]
            nc.tensor.matmul(rsB_ps, m4t_sb[:], rsb[:], start=True, stop=True)
            hh = HB // 2
            nc.vector.tensor_scalar_mul(out_sb[:, 0:hh], ex[:, 0:hh], rsB_ps)
            nc.sync.dma_start(out_d[:, 0:hh], out_sb[:, 0:hh])
            nc.vector.tensor_scalar_mul(out_sb[:, hh:HB], ex[:, hh:HB], rsB_ps)
            nc.scalar.dma_start(out_d[:, hh:HB], out_sb[:, hh:HB])

    nc.compile()
    return nc


def _get_nc():
    global _NC_CACHE
    if _NC_CACHE is None:
        _NC_CACHE = _build()
    return _NC_CACHE


def _prep_inputs(hidden, encoder_outputs, W_attn, b_attn, v):
    f = np.float32
    W_h = np.asarray(W_attn[:DH], dtype=f)
    W_e = np.asarray(W_attn[DH:], dtype=f)
    import ml_dtypes
    bf = ml_dtypes.bfloat16
    f8 = ml_dtypes.float8_e4m3
    wh_prep = np.clip(
        np.ascontiguousarray(W_h.reshape(KD, 128, H).transpose(1, 0, 2)) * WH_SCALE,
        -240.0, 240.0,
    ).astype(f8)
    we_prep = np.clip(
        np.ascontiguousarray(W_e.reshape(KH, 128, H).transpose(1, 0, 2)) * WE_SCALE,
        -240.0, 240.0,
    ).astype(f8)
    b_attn = np.asarray(b_attn, dtype=f)
    v = np.asarray(v, dtype=f)
    # ba4[p, q, j] = b_attn[q*128+p] replicated over the BC free dim
    ba_prep = np.ascontiguousarray(
        np.broadcast_to(b_attn.reshape(NQ, 128).T[:, :, None], (128, NQ, BC))
    ).astype(f)
    # vs32[p, q, c, j] = v[q*128+p] if j == c else 0
    vq = v.reshape(NQ, 128).T                      # [128, NQ]
    vs32 = np.zeros((128, NQ, NCH, 32), dtype=f)
    for c in range(NCH):
        vs32[:, :, c, c] = vq
    vs32_prep = vs32.astype(bf)
    # m4[32b+c, b] = 1 ; m4t[b, 32b+c] = 1
    m4 = np.zeros((128, BC), dtype=f)
    m4t = np.zeros((BC, 128), dtype=f)
    for b in range(BC):
        for c in range(NCH):
            m4[32 * b + c, b] = 1.0
            m4t[b, 32 * b + c] = 1.0
    ident = np.eye(BC, dtype=f)
    hidden = np.asarray(hidden, dtype=f)
    encoder_outputs = np.asarray(encoder_outputs, dtype=f)

    in_maps = []
    for core in range(NCORES):
        b0 = core * BC
        hc = hidden[b0 : b0 + BC]                       # [BC, DH]
        hid_prep = np.zeros((128, KD, 16), dtype=f8)
        hid_prep[:, :, 0:BC] = np.clip(
            np.ascontiguousarray(hc.T.reshape(KD, 128, BC).transpose(1, 0, 2)),
            -240.0, 240.0,
        ).astype(f8)
        ec = encoder_outputs[:, b0 : b0 + BC, :]        # [S, BC, H]
        # enc_prep[b, sblk, p, k, si] = ec[sblk*SBLK+si, b, k*128+p]
        enc_prep = np.clip(
            np.ascontiguousarray(
                ec.transpose(1, 0, 2)
                .reshape(BC, NBLK, SBLK, KH, 128)
                .transpose(0, 1, 4, 3, 2)
            ),
            -240.0, 240.0,
        ).astype(f8)
        in_maps.append(
            {
                "enc_t": enc_prep,
                "hid_t": hid_prep,
                "w_h": wh_prep,
                "w_e": we_prep,
                "ba4": ba_prep,
                "vs32": vs32_prep,
                "m4": m4,
                "m4t": m4t,
                "ident": ident,
            }
        )
    return in_maps


def _run(inputs, trace=False, **kw):
    nc = _get_nc()
    in_maps = _prep_inputs(
        inputs["hidden"],
        inputs["encoder_outputs"],
        inputs["W_attn"],
        inputs["b_attn"],
        inputs["v"],
    )
    res = run_bass_kernel_spmd(
        nc, in_maps, core_ids=list(range(NCORES)), trace=trace, **kw
    )
    # out_dev[32b+c, :] holds out[b, 512c : 512(c+1)]
    pieces = []
    for r in res.results:
        od = np.asarray(r["out"], dtype=np.float32)     # [128, 512]
        rows = od.reshape(BC, 32, HB)[:, 0:NCH, :]      # [BC, NCH, 512]
        pieces.append(rows.reshape(BC, S))
    out = np.concatenate(pieces, axis=0).astype(np.float32)
    return out, res


def kernel(**inputs):
    out, _ = _run(inputs, trace=False)
    return out
